# revision 17
# baseline (speedup 1.0000x reference)
"""DirSAGEConv Trainium2 kernel: 8-core SPMD gather + transposed one-hot scatter.

out = x @ Ws.T + bs + (1-a)*(mean_{src->dst}(x) @ W1.T + b1)
                    + a*(mean_{dst->src}(x) @ W2.T + b2)

Sharding: nodes (and the messages that scatter into them) are split into 8
contiguous blocks of 12500, one per NeuronCore; the x table (bf16) is
replicated in every core's DRAM.

v2 design notes (vs the per-128-window scheme):
- Scatter accumulates TRANSPOSED in PSUM: accT[feat, slot] over a 512-slot
  super-window (one PSUM bank), via matmul(lhsT=gathered_tile[msg, feat],
  rhs=one_hot[msg, slot_window]). Each 128-message tile lands at a host-
  chosen static 128-column offset o_t inside the bank, so messages only
  need padding to 128 per (dir, super-window, segment) REGION instead of
  per (dir, 128-window, segment) cell: ~425k slots/core vs ~453k.
- Measured SWDGE behavior (TRN2): the descriptor ring is 1024 descs/queue
  (calls above that die on HW regardless of dynamic_dma_scratch_size), the
  Pool exec queue holds 4 in-flight gather instructions (each occupying a
  slot until its DMA fully lands), and per-call turnaround is roughly
  1.6us fixed + ~6.3ns/desc per queue. 512-index calls on 4 round-robin
  queues (2 fit in the ring, 4 in flight) with bufs=3 pools measured best;
  span is pinned by SWDGE descriptor service (~2.6ns/desc aggregate),
  insensitive to gather locality (sequential-index probe: no change).
- The epilogue needs no PE transposes: accT slabs are exactly the lhsT the
  final 128x128 weight matmuls want. Mean scaling (1/cnt, per slot) happens
  after those matmuls via per-partition activation scale on [slot, feat]
  tiles, using separate PSUM regions for the x / m1 / m2 terms.
"""

import math
import sys
import types

import numpy as np

try:
    import ml_dtypes
except ImportError:  # pragma: no cover
    ml_dtypes = None

import concourse.bacc as bacc
import concourse.bass as bass
import concourse.mybir as mybir
import concourse.tile as tile
from concourse.bass_utils import run_bass_kernel_spmd

ALPHA = 0.5
NCORES = 8
P = 128          # partitions / feature dim
SW = 512         # slots per super-window (one PSUM bank of fp32)
import os as _os
_SP = _os.environ.get("GATHER_SP", "1") == "1"
_MC = int(_os.environ.get("GATHER_MAXCALL", "0")) or 512
_DDS = int(_os.environ.get("DMA_SCRATCH", "0")) or 16384
_GB = int(_os.environ.get("GPOOL_BUFS", "0")) or 3

BF16 = np.dtype(ml_dtypes.bfloat16) if ml_dtypes is not None else None


def _install_profile_hook():
    """Wire the NTFF profile hook trn_boot would install if antenv had
    axon_hooks (needed for trace=True exec_time_ns under axon)."""
    import antenv

    try:
        from antenv import axon_hooks  # noqa: F401

        return
    except ImportError:
        pass
    m = types.ModuleType("antenv.axon_hooks")
    m._hook = None
    m.set_axon_ntff_profile_hook = lambda h: setattr(m, "_hook", h)
    m.get_axon_ntff_profile_hook = lambda: m._hook
    sys.modules["antenv.axon_hooks"] = m
    antenv.axon_hooks = m
    try:
        if "/root/.axon_site" not in sys.path:
            sys.path.insert(0, "/root/.axon_site")
        from trn_agent_boot import trn_boot

        hook = trn_boot._ntff_profile_via_ctypes("/opt/axon/libaxon_pjrt.so")
        m.set_axon_ntff_profile_hook(hook)
    except Exception:
        pass


class Plan:
    """Static (core-uniform) message layout + per-core data arrays."""


def make_plan(edge_index, n_nodes, n_cores=NCORES):
    pl = Plan()
    npc = n_nodes // n_cores
    assert npc * n_cores == n_nodes
    nw = (npc + P - 1) // P           # 128-slot output windows per core
    nsw = (nw * P + SW - 1) // SW     # super-windows per core
    pl.n_nodes, pl.n_cores, pl.npc = n_nodes, n_cores, npc
    pl.nw, pl.nsw = nw, nsw
    pl.xrows = max(n_nodes, (n_cores - 1) * npc + nw * P)
    pl.xrows = (pl.xrows + P - 1) // P * P

    bounds = [0, 28000, 56000, 84000, pl.xrows]
    assert all(b - a <= 32768 for a, b in zip(bounds, bounds[1:]))
    nseg = len(bounds) - 1
    pl.nseg, pl.bounds = nseg, bounds

    src = np.ascontiguousarray(edge_index[0]).astype(np.int64)
    dst = np.ascontiguousarray(edge_index[1]).astype(np.int64)
    # direction 0: gather src, scatter dst (m_s2d); direction 1: the reverse
    g = np.concatenate([src, dst])
    s = np.concatenate([dst, src])
    d = np.repeat(np.array([0, 1], np.int64), src.shape[0])

    owner = s // npc
    sl = s - owner * npc
    swi = sl >> 9                     # super-window index
    slot = (sl & (SW - 1)).astype(np.int32)   # slot within super-window
    barr = np.asarray(bounds, np.int64)
    seg = np.searchsorted(barr, g, side="right") - 1
    gloc = (g - barr[seg]).astype(np.int16)
    cell = ((owner * 2 + d) * nsw + swi) * nseg + seg

    order = np.lexsort((g, sl, cell))
    cell_s = cell[order]
    gloc_s = gloc[order]
    slot_s = slot[order]

    ncells = n_cores * 2 * nsw * nseg
    counts = np.bincount(cell_s, minlength=ncells)
    cum = np.zeros(ncells + 1, np.int64)
    np.cumsum(counts, out=cum[1:])
    # region (d, sw, sg) size: max real count over cores, padded to 128
    Rreal = counts.reshape(n_cores, 2, nsw, nseg).max(axis=0)
    Rslots = (Rreal + P - 1) // P * P
    pl.Rslots = Rslots

    # slot/instance streams (identical layout for every core)
    # region order: (d, sw, sg)
    reg_off = {}
    slot_off = 0
    for di in range(2):
        for w in range(nsw):
            for sg in range(nseg):
                n = int(Rslots[di, w, sg])
                if n:
                    reg_off[(di, w, sg)] = (slot_off, n)
                    slot_off += n
    pl.reg_off = reg_off
    pl.total_slots = slot_off

    # per-core padded streams: gather idx + slot value (-1 = pad)
    gidx = np.zeros((n_cores, slot_off), np.int16)
    slotv = np.full((n_cores, slot_off), -1, np.int32)
    for c in range(n_cores):
        for (di, w, sg), (off, n) in reg_off.items():
            cid = ((c * 2 + di) * nsw + w) * nseg + sg
            a, b = cum[cid], cum[cid + 1]
            cnt = b - a
            if cnt:
                gidx[c, off:off + cnt] = gloc_s[a:b]
                slotv[c, off:off + cnt] = slot_s[a:b]

    # instance list: per region tile, one or more static column offsets o
    # such that every core's messages of that tile fit in [o, o+128).
    sw_actual = [min(SW, nw * P - w * SW) for w in range(nsw)]
    instances = {}   # (d, sw) -> list of (sg, t, o)  in emission order
    sval_cols = []   # list of [n_cores, 128] int16 sval columns
    sw_inst = {}     # (d, sw) -> (inst0, inst1)
    inst_ptr = 0
    for di in range(2):
        for w in range(nsw):
            swa = sw_actual[w]
            inst0 = inst_ptr
            lst = []
            for sg in range(nseg):
                if (di, w, sg) not in reg_off:
                    continue
                off, n = reg_off[(di, w, sg)]
                T = n // P
                sv = slotv[:, off:off + n].reshape(n_cores, T, P)
                for t in range(T):
                    tv = sv[:, t, :]                 # [C, 128]
                    valid = tv >= 0
                    if not valid.any():
                        # pure-padding tile: emit one dead instance
                        lst.append((sg, t, 0))
                        sval_cols.append(np.full((n_cores, P), -1, np.int16))
                        inst_ptr += 1
                        continue
                    smax = tv.max()
                    rem_lo = int(tv[valid].min())
                    covered = np.zeros_like(valid)
                    while True:
                        o = min(rem_lo, max(swa - P, 0))
                        inwin = valid & ~covered & (tv >= o) & (tv < o + P)
                        col = np.where(inwin, tv - o, -1).astype(np.int16)
                        lst.append((sg, t, o))
                        sval_cols.append(col)
                        inst_ptr += 1
                        covered |= inwin
                        left = valid & ~covered
                        if not left.any():
                            break
                        rem_lo = int(tv[left].min())
            instances[(di, w)] = lst
            sw_inst[(di, w)] = (inst0, inst_ptr)
    pl.instances, pl.sw_inst = instances, sw_inst
    pl.sw_actual = sw_actual
    pl.total_inst = inst_ptr

    # sval DRAM layout: [C, 128, total_inst]
    svarr = np.stack(sval_cols, axis=2)  # [C, 128, inst]
    pl.sv_dram = np.ascontiguousarray(svarr).astype(np.float32).astype(BF16)

    # dma_gather index layout: idx j of a call -> [j % 16, j // 16], tiled x8
    if _os.environ.get("GATHER_SEQIDX", "0") == "1":
        # perf probe: sequential indices (same desc count, ideal locality)
        for (di, w, sg), (off, n) in reg_off.items():
            gidx[:, off:off + n] = np.arange(n, dtype=np.int16)[None, :]
    gidx_dram = np.zeros((n_cores, P, slot_off // 16), np.int16)
    for (di, w, sg), (off, n) in reg_off.items():
        blk = gidx[:, off:off + n].reshape(n_cores, n // 16, 16)
        blk = blk.transpose(0, 2, 1)                       # [C, 16, n/16]
        gidx_dram[:, :, off // 16:(off + n) // 16] = np.tile(blk, (1, 8, 1))
    pl.gidx_dram = gidx_dram

    # reciprocal mean-normalizers: rtab[c, p, d*nw + w128] = 1/max(cnt, 1)
    win = sl >> 7
    wslot = ((owner * 2 + d) * nw + win) * P + (sl & 127)
    cntv = np.bincount(wslot, minlength=n_cores * 2 * nw * P)
    cntv = cntv.reshape(n_cores, 2, nw, P)
    rtab = (1.0 / np.maximum(cntv, 1)).astype(np.float32)
    pl.rtab = np.ascontiguousarray(rtab.transpose(0, 3, 1, 2).reshape(
        n_cores, P, 2 * nw))
    return pl


def build_program(pl, debug=False):
    dt = mybir.dt
    nc = bacc.Bacc("TRN2", target_bir_lowering=False, debug=debug,
                   num_devices=pl.n_cores, num_swdge_queues=4,
                   dynamic_dma_scratch_size=_DDS)
    nw, nsw, nseg = pl.nw, pl.nsw, pl.nseg
    xg = nc.dram_tensor("xg", [pl.xrows, P], dt.bfloat16, kind="ExternalInput")
    xbt = nc.dram_tensor("xbt", [P, nw * P], dt.bfloat16, kind="ExternalInput")
    gi = nc.dram_tensor("gi", [P, pl.total_slots // 16], dt.int16,
                        kind="ExternalInput")
    svd = nc.dram_tensor("svd", [P, pl.total_inst], dt.bfloat16,
                         kind="ExternalInput")
    iota = nc.dram_tensor("iota", [P, P], dt.bfloat16, kind="ExternalInput")
    zero = nc.dram_tensor("zero", [P, P], dt.bfloat16, kind="ExternalInput")
    rt = nc.dram_tensor("rt", [P, 2 * nw], dt.float32, kind="ExternalInput")
    onesrow = nc.dram_tensor("onesrow", [1, P], dt.bfloat16, kind="ExternalInput")
    wst = nc.dram_tensor("wst", [P, P], dt.bfloat16, kind="ExternalInput")
    w1t = nc.dram_tensor("w1t", [P, P], dt.bfloat16, kind="ExternalInput")
    w2t = nc.dram_tensor("w2t", [P, P], dt.bfloat16, kind="ExternalInput")
    btot = nc.dram_tensor("btot", [1, P], dt.bfloat16, kind="ExternalInput")
    outd = nc.dram_tensor("out", [nw * P, P], dt.bfloat16, kind="ExternalOutput")

    eq = mybir.AluOpType.is_equal
    add = mybir.AluOpType.add
    cp = mybir.ActivationFunctionType.Copy
    qrr = [0]
    nreg_cache = {}
    _PREP = _os.environ.get("GATHER_PREP", "0") == "1"
    gsems = [nc.alloc_semaphore(f"gsem{q}") for q in range(4)] if _PREP else None

    with tile.TileContext(nc) as tc:
        with (
            tc.tile_pool(name="const", bufs=1) as cpool,
            tc.tile_pool(name="gpool", bufs=_GB) as gpool,
            tc.tile_pool(name="ipool", bufs=_GB) as ipool,
            tc.tile_pool(name="svp", bufs=3) as svpool,
            tc.tile_pool(name="ohp", bufs=3) as ohpool,
            tc.tile_pool(name="sm", bufs=3) as smpool,
            tc.tile_pool(name="ab", bufs=3) as apool,
            tc.tile_pool(name="ob", bufs=4) as obpool,
            tc.tile_pool(name="acc", bufs=2, space="PSUM") as accpool,
            tc.tile_pool(name="opp", bufs=3, space="PSUM") as oppool,
            tc.tile_pool(name="iop", bufs=1, space="PSUM") as iopool,
        ):
            def cld(name, handle, shape):
                t = cpool.tile(shape, dt.bfloat16, tag=name)
                nc.scalar.dma_start(t[:], handle[:])
                return t

            iota_t = cld("iota", iota, [P, P])
            zero_t = cld("zero", zero, [P, P])
            iota_ps = iopool.tile([P, P], dt.float32, tag="iops")
            nc.scalar.activation(iota_ps[:], iota_t[:], cp)
            rt_t = cpool.tile([P, 2 * nw], dt.float32, tag="rt")
            nc.scalar.dma_start(rt_t[:], rt[:])
            onesrow_t = cld("onesrow", onesrow, [1, P])
            wst_t = cld("wst", wst, [P, P])
            w1t_t = cld("w1t", w1t, [P, P])
            w2t_t = cld("w2t", w2t, [P, P])
            btot_t = cld("btot", btot, [1, P])
            xbt_t = cld("xbt", xbt, [P, nw * P])
            m1_t = cpool.tile([P, nw * P], dt.bfloat16, tag="m1")

            for di in range(2):
                for w in range(nsw):
                    swa = pl.sw_actual[w]
                    i0, i1 = pl.sw_inst[(di, w)]
                    ninst = i1 - i0
                    inst = pl.instances[(di, w)]
                    # one-hot batch for the whole super-window
                    svt = svpool.tile([P, ninst], dt.bfloat16, tag="sv")
                    nc.scalar.dma_start(svt[:], svd[:, i0:i1])
                    oht = ohpool.tile([P, ninst, P], dt.bfloat16, tag="oh")
                    nc.vector.tensor_tensor(
                        oht[:],
                        iota_ps[:].unsqueeze(1).broadcast_to([P, ninst, P]),
                        svt[:].unsqueeze(2).broadcast_to([P, ninst, P]), eq)
                    # gather the super-window's regions (one call per region)
                    gts = {}
                    for sg in range(nseg):
                        if (di, w, sg) not in pl.reg_off:
                            continue
                        off, n = pl.reg_off[(di, w, sg)]
                        it = ipool.tile([P, n // 16], dt.int16, tag=f"i{sg}")
                        nc.sync.dma_start(it[:], gi[:, off // 16:(off + n) // 16])
                        gt = gpool.tile([P, n // P, P], dt.bfloat16,
                                        tag=f"g{sg}")
                        a, b = pl.bounds[sg], pl.bounds[sg + 1]
                        for c0 in range(0, n, _MC):
                            cn = min(_MC, n - c0)
                            if cn not in nreg_cache:
                                nreg_cache[cn] = nc.gpsimd.snap(cn)
                            q = qrr[0] % 4
                            if _PREP:
                                nc.gpsimd.dma_gather(
                                    gt[:, c0 // P:(c0 + cn) // P, :],
                                    xg[a:b, :],
                                    it[:, c0 // 16:(c0 + cn) // 16], cn,
                                    nreg_cache[cn], P, single_packet=_SP,
                                    queue_num=q, prepare_only=True,
                                    sem=gsems[q])
                                nc.gpsimd.trigger_dma(count=None, queue_num=q)
                            else:
                                nc.gpsimd.dma_gather(
                                    gt[:, c0 // P:(c0 + cn) // P, :],
                                    xg[a:b, :],
                                    it[:, c0 // 16:(c0 + cn) // 16], cn,
                                    nreg_cache[cn], P, single_packet=_SP,
                                    queue_num=q)
                            qrr[0] += 1
                        gts[sg] = gt
                    # transposed scatter-accumulate: accT[feat, slot]
                    acc = accpool.tile([P, SW], dt.float32, tag="acc")
                    nc.tensor.matmul(acc[:, 0:swa], lhsT=zero_t[:],
                                     rhs=xbt_t[:, 0:swa],
                                     start=True, stop=False)
                    for k, (sg, t, o) in enumerate(inst):
                        nc.tensor.matmul(
                            acc[:, o:o + P], lhsT=gts[sg][:, t, :],
                            rhs=oht[:, i0 - i0 + k, :],
                            start=False, stop=(k == ninst - 1))
                    # drain
                    if di == 0:
                        nc.scalar.activation(
                            m1_t[:, w * SW:w * SW + swa], acc[:, 0:swa], cp)
                    else:
                        mt = smpool.tile([P, swa], dt.bfloat16, tag="mt")
                        nc.scalar.activation(mt[:], acc[:, 0:swa], cp)
                        for wl in range(swa // P):
                            w128 = (w * SW) // P + wl
                            op = oppool.tile([P, 3 * P], dt.float32, tag="op")
                            nc.tensor.matmul(
                                op[:, 0:P],
                                lhsT=xbt_t[:, w128 * P:(w128 + 1) * P],
                                rhs=wst_t[:], start=True, stop=False)
                            nc.tensor.matmul(op[:, 0:P], lhsT=onesrow_t[:],
                                             rhs=btot_t[:], start=False,
                                             stop=True)
                            nc.tensor.matmul(
                                op[:, P:2 * P],
                                lhsT=m1_t[:, w128 * P:(w128 + 1) * P],
                                rhs=w1t_t[:], start=True, stop=True)
                            nc.tensor.matmul(op[:, 2 * P:3 * P],
                                             lhsT=mt[:, wl * P:(wl + 1) * P],
                                             rhs=w2t_t[:], start=True, stop=True)
                            a0 = apool.tile([P, P], dt.bfloat16, tag="a0")
                            nc.scalar.activation(a0[:], op[:, 0:P], cp)
                            a1 = apool.tile([P, P], dt.bfloat16, tag="a1")
                            nc.scalar.activation(a1[:], op[:, P:2 * P], cp,
                                                 scale=rt_t[:, w128:w128 + 1])
                            a2 = apool.tile([P, P], dt.bfloat16, tag="a2")
                            nc.scalar.activation(a2[:], op[:, 2 * P:3 * P], cp,
                                                 scale=rt_t[:, nw + w128:
                                                            nw + w128 + 1])
                            t12 = apool.tile([P, P], dt.bfloat16, tag="t12")
                            nc.vector.tensor_tensor(t12[:], a1[:], a2[:], add)
                            ob = obpool.tile([P, P], dt.bfloat16, tag="ob")
                            nc.vector.tensor_tensor(ob[:], t12[:], a0[:], add)
                            nc.scalar.dma_start(
                                outd[w128 * P:(w128 + 1) * P, :], ob[:])

    nc.compile()
    return nc


def make_inputs(pl, x, W1, b1, W2, b2, Ws, bs):
    """Per-core in_maps from the full inputs."""
    bf = BF16
    xpad = np.zeros((pl.xrows, P), np.float32)
    xpad[:pl.n_nodes] = np.asarray(x, np.float32)
    xg = xpad.astype(bf)
    iota = np.broadcast_to(np.arange(P, dtype=np.float32), (P, P))
    iota = np.ascontiguousarray(iota).astype(bf)
    zero = np.zeros((P, P), np.float32).astype(bf)
    onesrow = np.ones((1, P), np.float32).astype(bf)
    wst = np.ascontiguousarray(np.asarray(Ws, np.float32).T).astype(bf)
    w1t = np.ascontiguousarray(
        (1.0 - ALPHA) * np.asarray(W1, np.float32).T).astype(bf)
    w2t = np.ascontiguousarray(ALPHA * np.asarray(W2, np.float32).T).astype(bf)
    btot = (np.asarray(bs, np.float32) + (1.0 - ALPHA) * np.asarray(b1, np.float32)
            + ALPHA * np.asarray(b2, np.float32)).reshape(1, P).astype(bf)
    in_maps = []
    for c in range(pl.n_cores):
        xb = xg[c * pl.npc:c * pl.npc + pl.nw * P]
        in_maps.append({
            "xg": xg,
            "xbt": np.ascontiguousarray(xb.T),
            "gi": np.ascontiguousarray(pl.gidx_dram[c]),
            "svd": pl.sv_dram[c],
            "iota": iota,
            "zero": zero,
            "rt": pl.rtab[c], "onesrow": onesrow,
            "wst": wst, "w1t": w1t, "w2t": w2t, "btot": btot,
        })
    return in_maps


def kernel(x, edge_index, W1, b1, W2, b2, Ws, bs, _trace=False):
    x = np.asarray(x)
    n_nodes = x.shape[0]
    pl = make_plan(np.asarray(edge_index), n_nodes)
    nc = build_program(pl)
    in_maps = make_inputs(pl, x, W1, b1, W2, b2, Ws, bs)
    if _trace:
        _install_profile_hook()
    import os

    res = run_bass_kernel_spmd(nc, in_maps, core_ids=list(range(pl.n_cores)),
                               trace=_trace,
                               tmpdir=os.environ.get("BASS_TMPDIR") or None)
    out = np.empty((n_nodes, P), np.float32)
    for c in range(pl.n_cores):
        out[c * pl.npc:(c + 1) * pl.npc] = \
            res.results[c]["out"][:pl.npc].astype(np.float32)
    if _trace:
        kernel._last_exec_ns = res.exec_time_ns
        kernel._last_results = res
    return out


# revision 18
# speedup vs baseline: 1.0088x; 1.0088x over previous
"""DirSAGEConv Trainium2 kernel: 8-core SPMD gather + transposed one-hot scatter.

out = x @ Ws.T + bs + (1-a)*(mean_{src->dst}(x) @ W1.T + b1)
                    + a*(mean_{dst->src}(x) @ W2.T + b2)

Sharding: nodes (and the messages that scatter into them) are split into 8
contiguous blocks of 12500, one per NeuronCore; the x table (bf16) is
replicated in every core's DRAM.

v2 design notes (vs the per-128-window scheme):
- Scatter accumulates TRANSPOSED in PSUM: accT[feat, slot] over a 512-slot
  super-window (one PSUM bank), via matmul(lhsT=gathered_tile[msg, feat],
  rhs=one_hot[msg, slot_window]). Each 128-message tile lands at a host-
  chosen static 128-column offset o_t inside the bank, so messages only
  need padding to 128 per (dir, super-window, segment) REGION instead of
  per (dir, 128-window, segment) cell: ~425k slots/core vs ~453k.
- Measured SWDGE behavior (TRN2): the descriptor ring is 1024 descs/queue
  (calls above that die on HW regardless of dynamic_dma_scratch_size), the
  Pool exec queue holds 4 in-flight gather instructions (each occupying a
  slot until its DMA fully lands), and per-call turnaround is roughly
  1.6us fixed + ~6.3ns/desc per queue. 512-index calls on 4 round-robin
  queues (2 fit in the ring, 4 in flight) with bufs=3 pools measured best;
  span is pinned by SWDGE descriptor service (~2.6ns/desc aggregate),
  insensitive to gather locality (sequential-index probe: no change).
- The epilogue needs no PE transposes: accT slabs are exactly the lhsT the
  final 128x128 weight matmuls want. Mean scaling (1/cnt, per slot) happens
  after those matmuls via per-partition activation scale on [slot, feat]
  tiles, using separate PSUM regions for the x / m1 / m2 terms.
"""

import math
import sys
import types

import numpy as np

try:
    import ml_dtypes
except ImportError:  # pragma: no cover
    ml_dtypes = None

import concourse.bacc as bacc
import concourse.bass as bass
import concourse.mybir as mybir
import concourse.tile as tile
from concourse.bass_utils import run_bass_kernel_spmd

ALPHA = 0.5
NCORES = 8
P = 128          # partitions / feature dim
SW = 512         # slots per super-window (one PSUM bank of fp32)
import os as _os
_SP = _os.environ.get("GATHER_SP", "1") == "1"
_MC = int(_os.environ.get("GATHER_MAXCALL", "0")) or 512
_DDS = int(_os.environ.get("DMA_SCRATCH", "0")) or 16384
_GB = int(_os.environ.get("GPOOL_BUFS", "0")) or 3

BF16 = np.dtype(ml_dtypes.bfloat16) if ml_dtypes is not None else None


def _install_profile_hook():
    """Wire the NTFF profile hook trn_boot would install if antenv had
    axon_hooks (needed for trace=True exec_time_ns under axon)."""
    import antenv

    try:
        from antenv import axon_hooks  # noqa: F401

        return
    except ImportError:
        pass
    m = types.ModuleType("antenv.axon_hooks")
    m._hook = None
    m.set_axon_ntff_profile_hook = lambda h: setattr(m, "_hook", h)
    m.get_axon_ntff_profile_hook = lambda: m._hook
    sys.modules["antenv.axon_hooks"] = m
    antenv.axon_hooks = m
    try:
        if "/root/.axon_site" not in sys.path:
            sys.path.insert(0, "/root/.axon_site")
        from trn_agent_boot import trn_boot

        hook = trn_boot._ntff_profile_via_ctypes("/opt/axon/libaxon_pjrt.so")
        m.set_axon_ntff_profile_hook(hook)
    except Exception:
        pass


class Plan:
    """Static (core-uniform) message layout + per-core data arrays."""


def make_plan(edge_index, n_nodes, n_cores=NCORES):
    pl = Plan()
    npc = n_nodes // n_cores
    assert npc * n_cores == n_nodes
    nw = (npc + P - 1) // P           # 128-slot output windows per core
    nsw = (nw * P + SW - 1) // SW     # super-windows per core
    pl.n_nodes, pl.n_cores, pl.npc = n_nodes, n_cores, npc
    pl.nw, pl.nsw = nw, nsw
    pl.xrows = max(n_nodes, (n_cores - 1) * npc + nw * P)
    pl.xrows = (pl.xrows + P - 1) // P * P

    bounds = [0, 28000, 56000, 84000, pl.xrows]
    assert all(b - a <= 32768 for a, b in zip(bounds, bounds[1:]))
    nseg = len(bounds) - 1
    pl.nseg, pl.bounds = nseg, bounds

    src = np.ascontiguousarray(edge_index[0]).astype(np.int64)
    dst = np.ascontiguousarray(edge_index[1]).astype(np.int64)
    # direction 0: gather src, scatter dst (m_s2d); direction 1: the reverse
    g = np.concatenate([src, dst])
    s = np.concatenate([dst, src])
    d = np.repeat(np.array([0, 1], np.int64), src.shape[0])

    owner = s // npc
    sl = s - owner * npc
    swi = sl >> 9                     # super-window index
    slot = (sl & (SW - 1)).astype(np.int32)   # slot within super-window
    barr = np.asarray(bounds, np.int64)
    seg = np.searchsorted(barr, g, side="right") - 1
    gloc = (g - barr[seg]).astype(np.int16)
    cell = ((owner * 2 + d) * nsw + swi) * nseg + seg

    order = np.lexsort((g, sl, cell))
    cell_s = cell[order]
    gloc_s = gloc[order]
    slot_s = slot[order]

    ncells = n_cores * 2 * nsw * nseg
    counts = np.bincount(cell_s, minlength=ncells)
    cum = np.zeros(ncells + 1, np.int64)
    np.cumsum(counts, out=cum[1:])
    # region (d, sw, sg) size: max real count over cores, padded to 128
    Rreal = counts.reshape(n_cores, 2, nsw, nseg).max(axis=0)
    Rslots = (Rreal + P - 1) // P * P
    pl.Rslots = Rslots

    # slot/instance streams (identical layout for every core)
    # region order: (d, sw, sg)
    reg_off = {}
    slot_off = 0
    for di in range(2):
        for w in range(nsw):
            for sg in range(nseg):
                n = int(Rslots[di, w, sg])
                if n:
                    reg_off[(di, w, sg)] = (slot_off, n)
                    slot_off += n
    pl.reg_off = reg_off
    pl.total_slots = slot_off

    # per-core padded streams: gather idx + slot value (-1 = pad)
    gidx = np.zeros((n_cores, slot_off), np.int16)
    slotv = np.full((n_cores, slot_off), -1, np.int32)
    for c in range(n_cores):
        for (di, w, sg), (off, n) in reg_off.items():
            cid = ((c * 2 + di) * nsw + w) * nseg + sg
            a, b = cum[cid], cum[cid + 1]
            cnt = b - a
            if cnt:
                gidx[c, off:off + cnt] = gloc_s[a:b]
                slotv[c, off:off + cnt] = slot_s[a:b]

    # instance list: per region tile, one or more static column offsets o
    # such that every core's messages of that tile fit in [o, o+128).
    sw_actual = [min(SW, nw * P - w * SW) for w in range(nsw)]
    instances = {}   # (d, sw) -> list of (sg, t, o)  in emission order
    sval_cols = []   # list of [n_cores, 128] int16 sval columns
    sw_inst = {}     # (d, sw) -> (inst0, inst1)
    inst_ptr = 0
    for di in range(2):
        for w in range(nsw):
            swa = sw_actual[w]
            inst0 = inst_ptr
            lst = []
            for sg in range(nseg):
                if (di, w, sg) not in reg_off:
                    continue
                off, n = reg_off[(di, w, sg)]
                T = n // P
                sv = slotv[:, off:off + n].reshape(n_cores, T, P)
                for t in range(T):
                    tv = sv[:, t, :]                 # [C, 128]
                    valid = tv >= 0
                    if not valid.any():
                        # pure-padding tile: emit one dead instance
                        lst.append((sg, t, 0))
                        sval_cols.append(np.full((n_cores, P), -1, np.int16))
                        inst_ptr += 1
                        continue
                    smax = tv.max()
                    rem_lo = int(tv[valid].min())
                    covered = np.zeros_like(valid)
                    while True:
                        o = min(rem_lo, max(swa - P, 0))
                        inwin = valid & ~covered & (tv >= o) & (tv < o + P)
                        col = np.where(inwin, tv - o, -1).astype(np.int16)
                        lst.append((sg, t, o))
                        sval_cols.append(col)
                        inst_ptr += 1
                        covered |= inwin
                        left = valid & ~covered
                        if not left.any():
                            break
                        rem_lo = int(tv[left].min())
            instances[(di, w)] = lst
            sw_inst[(di, w)] = (inst0, inst_ptr)
    pl.instances, pl.sw_inst = instances, sw_inst
    pl.sw_actual = sw_actual
    pl.total_inst = inst_ptr

    # sval DRAM layout: [C, 128, total_inst]
    svarr = np.stack(sval_cols, axis=2)  # [C, 128, inst]
    pl.sv_dram = np.ascontiguousarray(svarr).astype(np.float32).astype(BF16)

    # dma_gather index layout: idx j of a call -> [j % 16, j // 16], tiled x8
    if _os.environ.get("GATHER_SEQIDX", "0") == "1":
        # perf probe: sequential indices (same desc count, ideal locality)
        for (di, w, sg), (off, n) in reg_off.items():
            gidx[:, off:off + n] = np.arange(n, dtype=np.int16)[None, :]
    gidx_dram = np.zeros((n_cores, P, slot_off // 16), np.int16)
    for (di, w, sg), (off, n) in reg_off.items():
        blk = gidx[:, off:off + n].reshape(n_cores, n // 16, 16)
        blk = blk.transpose(0, 2, 1)                       # [C, 16, n/16]
        gidx_dram[:, :, off // 16:(off + n) // 16] = np.tile(blk, (1, 8, 1))
    pl.gidx_dram = gidx_dram

    # reciprocal mean-normalizers: rtab[c, p, d*nw + w128] = 1/max(cnt, 1)
    win = sl >> 7
    wslot = ((owner * 2 + d) * nw + win) * P + (sl & 127)
    cntv = np.bincount(wslot, minlength=n_cores * 2 * nw * P)
    cntv = cntv.reshape(n_cores, 2, nw, P)
    rtab = (1.0 / np.maximum(cntv, 1)).astype(np.float32)
    pl.rtab = np.ascontiguousarray(rtab.transpose(0, 3, 1, 2).reshape(
        n_cores, P, 2 * nw))
    return pl


def build_program(pl, debug=False):
    dt = mybir.dt
    nc = bacc.Bacc("TRN2", target_bir_lowering=False, debug=debug,
                   num_devices=pl.n_cores, num_swdge_queues=4,
                   dynamic_dma_scratch_size=_DDS)
    nw, nsw, nseg = pl.nw, pl.nsw, pl.nseg
    xg = nc.dram_tensor("xg", [pl.xrows, P], dt.bfloat16, kind="ExternalInput")
    xbt = nc.dram_tensor("xbt", [P, nw * P], dt.bfloat16, kind="ExternalInput")
    gi = nc.dram_tensor("gi", [P, pl.total_slots // 16], dt.int16,
                        kind="ExternalInput")
    svd = nc.dram_tensor("svd", [P, pl.total_inst], dt.bfloat16,
                         kind="ExternalInput")
    iota = nc.dram_tensor("iota", [P, P], dt.bfloat16, kind="ExternalInput")
    zero = nc.dram_tensor("zero", [P, P], dt.bfloat16, kind="ExternalInput")
    rt = nc.dram_tensor("rt", [P, 2 * nw], dt.float32, kind="ExternalInput")
    onesrow = nc.dram_tensor("onesrow", [1, P], dt.bfloat16, kind="ExternalInput")
    wst = nc.dram_tensor("wst", [P, P], dt.bfloat16, kind="ExternalInput")
    w1t = nc.dram_tensor("w1t", [P, P], dt.bfloat16, kind="ExternalInput")
    w2t = nc.dram_tensor("w2t", [P, P], dt.bfloat16, kind="ExternalInput")
    btot = nc.dram_tensor("btot", [1, P], dt.bfloat16, kind="ExternalInput")
    outd = nc.dram_tensor("out", [nw * P, P], dt.bfloat16, kind="ExternalOutput")

    eq = mybir.AluOpType.is_equal
    add = mybir.AluOpType.add
    cp = mybir.ActivationFunctionType.Copy
    qrr = [0]
    nreg_cache = {}
    _PREP = _os.environ.get("GATHER_PREP", "0") == "1"
    gsems = [nc.alloc_semaphore(f"gsem{q}") for q in range(4)] if _PREP else None

    with tile.TileContext(nc) as tc:
        with (
            tc.tile_pool(name="const", bufs=1) as cpool,
            tc.tile_pool(name="gpool", bufs=_GB) as gpool,
            tc.tile_pool(name="ipool", bufs=_GB) as ipool,
            tc.tile_pool(name="svp", bufs=3) as svpool,
            tc.tile_pool(name="ohp", bufs=3) as ohpool,
            tc.tile_pool(name="sm", bufs=2) as smpool,
            tc.tile_pool(name="ab", bufs=2) as apool,
            tc.tile_pool(name="ob", bufs=3) as obpool,
            tc.tile_pool(name="acc", bufs=2, space="PSUM") as accpool,
            tc.tile_pool(name="opp", bufs=2, space="PSUM") as oppool,
            tc.tile_pool(name="iop", bufs=1, space="PSUM") as iopool,
        ):
            def cld(name, handle, shape):
                t = cpool.tile(shape, dt.bfloat16, tag=name)
                nc.scalar.dma_start(t[:], handle[:])
                return t

            iota_t = cld("iota", iota, [P, P])
            zero_t = cld("zero", zero, [P, P])
            iota_ps = iopool.tile([P, P], dt.float32, tag="iops")
            nc.scalar.activation(iota_ps[:], iota_t[:], cp)
            rt_t = cpool.tile([P, 2 * nw], dt.float32, tag="rt")
            nc.scalar.dma_start(rt_t[:], rt[:])
            onesrow_t = cld("onesrow", onesrow, [1, P])
            wst_t = cld("wst", wst, [P, P])
            w1t_t = cld("w1t", w1t, [P, P])
            w2t_t = cld("w2t", w2t, [P, P])
            btot_t = cld("btot", btot, [1, P])
            xbt_t = cld("xbt", xbt, [P, nw * P])
            m1_t = cpool.tile([P, nw * P], dt.bfloat16, tag="m1")

            for di in range(2):
                for w in range(nsw):
                    swa = pl.sw_actual[w]
                    i0, i1 = pl.sw_inst[(di, w)]
                    ninst = i1 - i0
                    inst = pl.instances[(di, w)]
                    # one-hot batch for the whole super-window
                    svt = svpool.tile([P, ninst], dt.bfloat16, tag="sv")
                    nc.scalar.dma_start(svt[:], svd[:, i0:i1])
                    oht = ohpool.tile([P, ninst, P], dt.bfloat16, tag="oh")
                    nc.vector.tensor_tensor(
                        oht[:],
                        iota_ps[:].unsqueeze(1).broadcast_to([P, ninst, P]),
                        svt[:].unsqueeze(2).broadcast_to([P, ninst, P]), eq)
                    # gather the super-window's regions (one call per region)
                    gts = {}
                    for sg in range(nseg):
                        if (di, w, sg) not in pl.reg_off:
                            continue
                        off, n = pl.reg_off[(di, w, sg)]
                        it = ipool.tile([P, n // 16], dt.int16, tag=f"i{sg}")
                        nc.sync.dma_start(it[:], gi[:, off // 16:(off + n) // 16])
                        gt = gpool.tile([P, n // P, P], dt.bfloat16,
                                        tag=f"g{sg}")
                        a, b = pl.bounds[sg], pl.bounds[sg + 1]
                        for c0 in range(0, n, _MC):
                            cn = min(_MC, n - c0)
                            if cn not in nreg_cache:
                                nreg_cache[cn] = nc.gpsimd.snap(cn)
                            q = qrr[0] % 4
                            if _PREP:
                                nc.gpsimd.dma_gather(
                                    gt[:, c0 // P:(c0 + cn) // P, :],
                                    xg[a:b, :],
                                    it[:, c0 // 16:(c0 + cn) // 16], cn,
                                    nreg_cache[cn], P, single_packet=_SP,
                                    queue_num=q, prepare_only=True,
                                    sem=gsems[q])
                                nc.gpsimd.trigger_dma(count=None, queue_num=q)
                            else:
                                nc.gpsimd.dma_gather(
                                    gt[:, c0 // P:(c0 + cn) // P, :],
                                    xg[a:b, :],
                                    it[:, c0 // 16:(c0 + cn) // 16], cn,
                                    nreg_cache[cn], P, single_packet=_SP,
                                    queue_num=q)
                            qrr[0] += 1
                        gts[sg] = gt
                    # transposed scatter-accumulate: accT[feat, slot]
                    acc = accpool.tile([P, SW], dt.float32, tag="acc")
                    nc.tensor.matmul(acc[:, 0:swa], lhsT=zero_t[:],
                                     rhs=xbt_t[:, 0:swa],
                                     start=True, stop=False)
                    for k, (sg, t, o) in enumerate(inst):
                        nc.tensor.matmul(
                            acc[:, o:o + P], lhsT=gts[sg][:, t, :],
                            rhs=oht[:, i0 - i0 + k, :],
                            start=False, stop=(k == ninst - 1))
                    # drain
                    if di == 0:
                        nc.scalar.activation(
                            m1_t[:, w * SW:w * SW + swa], acc[:, 0:swa], cp)
                    else:
                        mt = smpool.tile([P, swa], dt.bfloat16, tag="mt")
                        nc.scalar.activation(mt[:], acc[:, 0:swa], cp)
                        for wl in range(swa // P):
                            w128 = (w * SW) // P + wl
                            op = oppool.tile([P, 3 * P], dt.float32, tag="op")
                            nc.tensor.matmul(
                                op[:, 0:P],
                                lhsT=xbt_t[:, w128 * P:(w128 + 1) * P],
                                rhs=wst_t[:], start=True, stop=False)
                            nc.tensor.matmul(op[:, 0:P], lhsT=onesrow_t[:],
                                             rhs=btot_t[:], start=False,
                                             stop=True)
                            nc.tensor.matmul(
                                op[:, P:2 * P],
                                lhsT=m1_t[:, w128 * P:(w128 + 1) * P],
                                rhs=w1t_t[:], start=True, stop=True)
                            nc.tensor.matmul(op[:, 2 * P:3 * P],
                                             lhsT=mt[:, wl * P:(wl + 1) * P],
                                             rhs=w2t_t[:], start=True, stop=True)
                            a0 = apool.tile([P, P], dt.bfloat16, tag="a0")
                            nc.scalar.activation(a0[:], op[:, 0:P], cp)
                            a1 = apool.tile([P, P], dt.bfloat16, tag="a1")
                            nc.scalar.activation(a1[:], op[:, P:2 * P], cp,
                                                 scale=rt_t[:, w128:w128 + 1])
                            a2 = apool.tile([P, P], dt.bfloat16, tag="a2")
                            nc.scalar.activation(a2[:], op[:, 2 * P:3 * P], cp,
                                                 scale=rt_t[:, nw + w128:
                                                            nw + w128 + 1])
                            t12 = apool.tile([P, P], dt.bfloat16, tag="t12")
                            nc.vector.tensor_tensor(t12[:], a1[:], a2[:], add)
                            ob = obpool.tile([P, P], dt.bfloat16, tag="ob")
                            nc.vector.tensor_tensor(ob[:], t12[:], a0[:], add)
                            nc.scalar.dma_start(
                                outd[w128 * P:(w128 + 1) * P, :], ob[:])

    nc.compile()
    return nc


def make_inputs(pl, x, W1, b1, W2, b2, Ws, bs):
    """Per-core in_maps from the full inputs."""
    bf = BF16
    xpad = np.zeros((pl.xrows, P), np.float32)
    xpad[:pl.n_nodes] = np.asarray(x, np.float32)
    xg = xpad.astype(bf)
    iota = np.broadcast_to(np.arange(P, dtype=np.float32), (P, P))
    iota = np.ascontiguousarray(iota).astype(bf)
    zero = np.zeros((P, P), np.float32).astype(bf)
    onesrow = np.ones((1, P), np.float32).astype(bf)
    wst = np.ascontiguousarray(np.asarray(Ws, np.float32).T).astype(bf)
    w1t = np.ascontiguousarray(
        (1.0 - ALPHA) * np.asarray(W1, np.float32).T).astype(bf)
    w2t = np.ascontiguousarray(ALPHA * np.asarray(W2, np.float32).T).astype(bf)
    btot = (np.asarray(bs, np.float32) + (1.0 - ALPHA) * np.asarray(b1, np.float32)
            + ALPHA * np.asarray(b2, np.float32)).reshape(1, P).astype(bf)
    in_maps = []
    for c in range(pl.n_cores):
        xb = xg[c * pl.npc:c * pl.npc + pl.nw * P]
        in_maps.append({
            "xg": xg,
            "xbt": np.ascontiguousarray(xb.T),
            "gi": np.ascontiguousarray(pl.gidx_dram[c]),
            "svd": pl.sv_dram[c],
            "iota": iota,
            "zero": zero,
            "rt": pl.rtab[c], "onesrow": onesrow,
            "wst": wst, "w1t": w1t, "w2t": w2t, "btot": btot,
        })
    return in_maps


def kernel(x, edge_index, W1, b1, W2, b2, Ws, bs, _trace=False):
    x = np.asarray(x)
    n_nodes = x.shape[0]
    pl = make_plan(np.asarray(edge_index), n_nodes)
    nc = build_program(pl)
    in_maps = make_inputs(pl, x, W1, b1, W2, b2, Ws, bs)
    if _trace:
        _install_profile_hook()
    import os

    res = run_bass_kernel_spmd(nc, in_maps, core_ids=list(range(pl.n_cores)),
                               trace=_trace,
                               tmpdir=os.environ.get("BASS_TMPDIR") or None)
    out = np.empty((n_nodes, P), np.float32)
    for c in range(pl.n_cores):
        out[c * pl.npc:(c + 1) * pl.npc] = \
            res.results[c]["out"][:pl.npc].astype(np.float32)
    if _trace:
        kernel._last_exec_ns = res.exec_time_ns
        kernel._last_results = res
    return out
